# revision 20
# baseline (speedup 1.0000x reference)
"""Multi-head causal attention (B=4, S=2048, D=1024, H=16) on 8 TRN2 NeuronCores.

Sharding: core c handles batch b = c//2 and head-group hg = c%2 (8 heads each).
Each core computes Q/K/V projections for its (batch, head-group), causal
attention, and a partial output projection over its 512 head-dims.  The host
sums the two partials per batch and adds b_o.  No collectives.

Device-side layout choices:
  - x is passed transposed (xT [D, S]) so projection matmuls contract over
    partitions directly.  x and all weights are DMA'd ONCE into SBUF with
    consolidated 3D-AP transfers and cached for the whole kernel.
  - Q and K are produced transposed (QT/KT [dq, S]); scores are computed
    transposed (S^T [kpos, q]) which makes the softmax denominator a matmul
    with a ones-column (no partition reductions anywhere).
  - V tiles are [128 tok, 9, 64]: blocks 0-7 hold the 8 heads' V, block 8 is
    a shared ones-block.  The AV stationary for head h is the strided slice
    [:, h:9:(8-h), :] = [V_h | ones], so psum rows 0-63 get U^T and rows
    64-127 the softmax denominator, at no extra PE column cost.
  - AV matmuls are causally trimmed: P^T columns q < kt*128 are identically
    zero and are never computed, exp'd, or streamed.
  - No max-subtraction in softmax: scaled scores are ~N(0,1), exp is safe.
  - Scheduling: a work queue of single-matmul "filler" steps (projection
    chains for future blocks, V chains, output projections) is drained inside
    the attention kt-loop, so the PE always has ACT-independent work while
    the scalar engine chews through the exps.
"""

import sys
import os

sys.path.insert(0, "/opt/trn_rl_repo")

from collections import deque

import numpy as np

import concourse.bacc as bacc
import concourse.mybir as mybir
import concourse.tile as tile
from concourse.bass_utils import run_bass_kernel_spmd

# The ACT table-load pass resolves each activation to the first table set
# containing it, which puts Exp (exp_and_others) and Ln
# (natural_log_exp_and_others) in different sets and reloads tables at every
# softmax normalization.  Restrict Exp/Ln to the one set that holds both so
# the whole kernel runs off a single table load.
_orig_get_tables = bacc.get_activation_tables


def _patched_tables(arch):
    t = _orig_get_tables(arch)
    for name, fns in t.items():
        if name != "natural_log_exp_and_others":
            fns.discard(mybir.ActivationFunctionType.Exp)
            fns.discard(mybir.ActivationFunctionType.Ln)
    return t


bacc.get_activation_tables = _patched_tables

B, S, D, H = 4, 2048, 1024, 16
DK = D // H          # 64
HH = H // 2          # 8 heads per core
HD = HH * DK         # 512 head-dims per core
N_CORES = 8

F32 = mybir.dt.float32
F16 = mybir.dt.float16

SCALE = 1.0 / np.sqrt(DK)


def build_nc(s=S):
    """Build the per-core SPMD program.  `s` is the sequence length (tunable
    for small-scale simulation; must be a multiple of 512)."""
    assert s % 512 == 0
    n_qb = s // 512          # 512-wide q blocks
    n_t128 = s // 128        # 128-wide token tiles
    n_dt = D // 128          # din tiles (8)
    n_dq = HD // 128         # dq tiles (4)

    nc = bacc.Bacc("TRN2", target_bir_lowering=False, debug=False,
                   num_devices=N_CORES)

    xT = nc.dram_tensor("xT", [D, s], F16, kind="ExternalInput")
    wqT = nc.dram_tensor("wqT", [D, HD], F16, kind="ExternalInput")
    wkT = nc.dram_tensor("wkT", [D, HD], F16, kind="ExternalInput")
    wvT = nc.dram_tensor("wvT", [D, HD], F16, kind="ExternalInput")
    woT = nc.dram_tensor("woT", [HD, D], F16, kind="ExternalInput")
    out = nc.dram_tensor("out", [s, D], F32, kind="ExternalOutput")

    with tile.TileContext(nc) as tc:
        with tc.tile_pool(name="persist", bufs=1) as persist, \
             tc.tile_pool(name="xc", bufs=3) as xc_pool, \
             tc.tile_pool(name="pT", bufs=40) as pT_pool, \
             tc.tile_pool(name="aoT", bufs=8) as aoT_pool, \
             tc.tile_pool(name="rb", bufs=3) as rb_pool, \
             tc.tile_pool(name="outsb", bufs=2) as out_pool, \
             tc.tile_pool(name="spsum", bufs=3, space="PSUM") as spsum, \
             tc.tile_pool(name="upsum", bufs=3, space="PSUM") as upsum, \
             tc.tile_pool(name="opsum", bufs=2, space="PSUM") as opsum:

            # Persistent SBUF arrays (live for the whole kernel).
            qt_sb = [persist.tile([128, s], F16, tag=f"qt{d}", name=f"qt{d}")
                     for d in range(n_dq)]
            # Per-head K^T tiles, zero-padded to 128 contraction rows: head h
            # occupies rows (h%2)*64..(h%2)*64+63, the other 64 rows are zero.
            # Scores matmuls can then use full 128x128 PE mode (the zero rows
            # multiply the paired head's Q rows harmlessly) -- avoiding tiling
            # mode switches, which drain the PE between instructions.
            kt_sb = [persist.tile([128, s], F16, tag=f"kt{h}", name=f"kt{h}")
                     for h in range(HH)]
            for h in range(HH):
                z = (1 - h % 2) * 64
                nc.vector.memset(kt_sb[h][z:z + 64, :], 0.0)
            # V tiles hold [t, head, 2*dk]: cols 0-63 are V, cols 64-127 are
            # 1.0.  As the AV stationary this makes the matmul emit U^T on
            # psum rows 0-63 and the softmax denominator on rows 64-127.
            v_sb = [persist.tile([128, HH, 2 * DK], F16, tag=f"v{t}",
                                 name=f"v{t}") for t in range(n_t128)]
            for t in range(n_t128):
                nc.vector.memset(v_sb[t][:, :, DK:2 * DK], 1.0)
            wq_sb = persist.tile([128, n_dt, HD], F16, tag="wq", name="wq")
            wk_sb = persist.tile([128, n_dt, HD], F16, tag="wk", name="wk")
            wv_sb = persist.tile([128, n_dt, HD], F16, tag="wv", name="wv")
            wo_sb = persist.tile([128, n_dq, D], F16, tag="wo", name="wo")

            # Warmup fodder: the PE clock ramps with sustained utilization
            # (p-states 0.65/1.2/2.4 GHz).  During the DMA-paced startup the
            # PE idles in bursts and gets stuck at the mid p-state, running
            # early matmuls at ~2.8x their steady-state time.  Dummy
            # LDWEIGHTS (no psum, no hazards -- every real matmul self-loads
            # its stationary) keep utilization up through that window.
            dmy = persist.tile([128, 128], F16, tag="dmy", name="dmy")
            nc.vector.memset(dmy[:], 0.0)

            def warm(n):
                for _ in range(n):
                    nc.tensor.ldweights(weights=dmy[:])

            xr = xT[:].rearrange("(i p) t -> p i t", p=128)
            wqr = wqT[:].rearrange("(i p) f -> p i f", p=128)
            wkr = wkT[:].rearrange("(i p) f -> p i f", p=128)
            wvr = wvT[:].rearrange("(i p) f -> p i f", p=128)
            wor = woT[:].rearrange("(i p) f -> p i f", p=128)

            # x block cache: 3 rotating slots (2 blocks live at a time plus
            # prefetch); tiles allocated when their DMA is issued.
            xc = {}

            def load_x_block(b):
                xc[b] = xc_pool.tile([128, n_dt, 512], F16, tag="xc",
                                     name=f"xcb{b}")
                nc.scalar.dma_start(out=xc[b][:],
                                    in_=xr[:, :, b * 512:(b + 1) * 512])

            # Startup-critical DMAs interleaved per din tile on the sync
            # queue: the first Q/K chains start as soon as their inputs land.
            xc[0] = xc_pool.tile([128, n_dt, 512], F16, tag="xc", name="xcb0")
            for i in range(n_dt):
                nc.sync.dma_start(out=xc[0][:, i, :], in_=xr[:, i, 0:512])
                nc.sync.dma_start(out=wq_sb[:, i, :], in_=wqr[:, i, :])
                nc.sync.dma_start(out=wk_sb[:, i, :], in_=wkr[:, i, :])
            # Non-critical weights + x blocks 1-2 on the ACT hwdge queue.
            nc.scalar.dma_start(out=wv_sb[:], in_=wvr)
            nc.scalar.dma_start(out=wo_sb[:], in_=wor)
            for b in (1, 2):
                if b < n_qb:
                    load_x_block(b)

            # ---- PE work generators (one matmul per yield) ----

            def gen_qk_chain(wsb, dq, tb, is_k):
                """Q^T or K^T projection for one 512-token block, one dq."""
                ps = opsum.tile([128, 512], F32, tag="op", name="pp")
                for i in range(n_dt):
                    nc.tensor.matmul(
                        ps[:],
                        lhsT=wsb[:, i, dq * 128:(dq + 1) * 128],
                        rhs=xc[tb][:, i, :],
                        start=(i == 0), stop=(i == n_dt - 1),
                    )
                    if i < n_dt - 1:
                        yield
                if is_k:
                    for e in (0, 1):
                        nc.vector.tensor_copy(
                            out=kt_sb[2 * dq + e][e * 64:(e + 1) * 64,
                                                  tb * 512:(tb + 1) * 512],
                            in_=ps[e * 64:(e + 1) * 64, :])
                else:
                    nc.vector.tensor_copy(
                        out=qt_sb[dq][:, tb * 512:(tb + 1) * 512], in_=ps[:])

            def gen_v_chain(t):
                """V projection for one 128-token tile."""
                tb, off = t // 4, (t % 4) * 128
                vp = opsum.tile([128, 512], F32, tag="op", name="vp")
                for i in range(n_dt):
                    nc.tensor.matmul(
                        vp[:],
                        lhsT=xc[tb][:, i, off:off + 128],
                        rhs=wv_sb[:, i, :],
                        start=(i == 0), stop=(i == n_dt - 1),
                    )
                    if i < n_dt - 1:
                        yield
                nc.vector.tensor_copy(
                    out=v_sb[t][:, :, 0:DK],
                    in_=vp[:].rearrange("p (h k) -> p h k", h=HH))

            def gen_oproj(qb, aos):
                """Output projection for q-block qb from its 4 ao tiles."""
                for qt_l in range(4):
                    qt = 4 * qb + qt_l
                    osb = out_pool.tile([128, D], F32, tag="osb", name="osb")
                    for half in range(2):
                        op = opsum.tile([128, 512], F32, tag="op", name="op")
                        for hp in range(n_dq):
                            nc.tensor.matmul(
                                op[:],
                                lhsT=aos[hp][:, qt_l * 128:(qt_l + 1) * 128],
                                rhs=wo_sb[:, hp, half * 512:(half + 1) * 512],
                                start=(hp == 0), stop=(hp == n_dq - 1),
                            )
                            if not (half == 1 and hp == n_dq - 1):
                                yield
                        nc.vector.tensor_copy(
                            out=osb[:, half * 512:(half + 1) * 512], in_=op[:])
                    nc.sync.dma_start(
                        out=out[qt * 128:(qt + 1) * 128, :], in_=osb[:])
                    yield

            workq = deque()

            def fill(n):
                done = 0
                while done < n and workq:
                    try:
                        next(workq[0])
                        done += 1
                    except StopIteration:
                        workq.popleft()

            def run_now(g):
                for _ in g:
                    pass

            # ---- attention ----

            def emit_score_kt(qb, hp, kt, pT):
                lo = max(kt - 4 * qb, 0) * 128
                w = 512 - lo
                for hh in (0, 1):
                    sp = spsum.tile([128, 512], F32, tag="sp", name="sp")
                    nc.tensor.matmul(
                        sp[:, lo:512],
                        lhsT=kt_sb[2 * hp + hh][:, kt * 128:(kt + 1) * 128],
                        rhs=qt_sb[hp][:, qb * 512 + lo:(qb + 1) * 512],
                        start=True, stop=True,
                    )
                    p = pT_pool.tile([128, w], F16, tag="p", name="p")
                    nc.scalar.activation(
                        out=p[:], in_=sp[:, lo:512],
                        func=mybir.ActivationFunctionType.Exp,
                        scale=float(SCALE))
                    if kt >= 4 * qb:
                        # zero strict-upper (kpos > q) region of the
                        # diagonal-crossing tile
                        nc.gpsimd.affine_select(
                            out=p[:], in_=p[:],
                            compare_op=mybir.AluOpType.is_ge,
                            fill=0.0, base=0, channel_multiplier=-1,
                            pattern=[[1, w]])
                    pT[(kt, hh)] = (p, lo)

            def emit_pair(cur, nxt, pT_cur, pT_next, per_kt):
                """Interleave next pair's scores with current pair's AV
                chains at kt granularity, plus `per_kt` filler steps: the PE
                gets dependency-free matmuls to run while the ACT engine
                works through the scores' exps."""
                nkt_cur = 4 * cur[0] + 4 if cur else 0
                nkt_nxt = 4 * nxt[0] + 4 if nxt else 0
                u = {}
                ao = None
                if cur:
                    ao = aoT_pool.tile([128, 512], F16, tag="aoT", name="aoT")
                    for hh in (0, 1):
                        u[hh] = upsum.tile([128, 512], F32, tag="u", name="u")
                for kt in range(max(nkt_cur, nkt_nxt)):
                    if kt < nkt_nxt:
                        emit_score_kt(nxt[0], nxt[1], kt, pT_next)
                    if kt == 0:
                        # fillers first: the pair's first AV matmul may wait
                        # on the u-slot WAR (previous pair's normalize chain);
                        # let the PE chew fillers during that window.
                        fill(per_kt)
                    if kt < nkt_cur:
                        for hh in (0, 1):
                            p, lo = pT_cur[(kt, hh)]
                            nc.tensor.matmul(
                                u[hh][:, lo:512],
                                lhsT=v_sb[kt][:, 2 * cur[1] + hh, :],
                                rhs=p[:],
                                start=(kt == 0), stop=(kt == nkt_cur - 1),
                            )
                    if kt > 0:
                        fill(per_kt)
                if cur:
                    # Both heads' denominators (u rows 64-127) staged into one
                    # rb tile so a single Ln+Exp pass computes 1/l for both.
                    # 1/l = exp(-ln(l)): ln and exp share one ACT table set,
                    # so no table reloads.
                    rb = rb_pool.tile([128, 512], F32, tag="rb", name="rb")
                    for hh in (0, 1):
                        nc.vector.tensor_copy(
                            out=rb[hh * 64:(hh + 1) * 64, :],
                            in_=u[hh][64:128, :])
                    nc.scalar.activation(
                        out=rb[:], in_=rb[:],
                        func=mybir.ActivationFunctionType.Ln)
                    nc.scalar.activation(
                        out=rb[:], in_=rb[:],
                        func=mybir.ActivationFunctionType.Exp, scale=-1.0)
                    for hh in (0, 1):
                        nc.vector.tensor_mul(
                            out=ao[hh * 64:(hh + 1) * 64, :],
                            in0=u[hh][0:64, :], in1=rb[hh * 64:(hh + 1) * 64, :])
                return ao

            # ---- schedule ----
            # Pre-loop: block-0 Q/K chains (dq-interleaved so pair (0,0)
            # unblocks after two chains), pair-(0,0) scores, block-0 V
            # chains, block-1 Q/K chains.  Everything later flows through
            # the filler queue.  warm() calls plug the DMA-paced idle gaps
            # so the PE clock ramps and stays at full speed.
            warm(40)
            for dq in range(n_dq):
                run_now(gen_qk_chain(wq_sb, dq, 0, False))
                warm(5)
                run_now(gen_qk_chain(wk_sb, dq, 0, True))
                warm(5)
            pairs = [(qb, hp) for qb in range(n_qb) for hp in range(n_dq)]
            pT_next = {}
            emit_pair(None, pairs[0], None, pT_next, 0)
            for t in range(min(4, n_t128)):
                run_now(gen_v_chain(t))
                warm(3)
            if n_qb > 1:
                for dq in range(n_dq):
                    run_now(gen_qk_chain(wq_sb, dq, 1, False))
                    run_now(gen_qk_chain(wk_sb, dq, 1, True))

            ao_by_qb = {qb: [] for qb in range(n_qb)}
            for i, (qb, hp) in enumerate(pairs):
                pT_cur, pT_next = pT_next, {}
                nxt = pairs[i + 1] if i + 1 < len(pairs) else None
                if hp == 0:
                    if qb + 3 < n_qb:
                        load_x_block(qb + 3)
                    if qb > 0:
                        workq.append(gen_oproj(qb - 1, ao_by_qb.pop(qb - 1)))
                    if qb + 2 < n_qb:
                        for dq in range(n_dq):
                            workq.append(gen_qk_chain(wq_sb, dq, qb + 2, False))
                            workq.append(gen_qk_chain(wk_sb, dq, qb + 2, True))
                if hp == 2 and qb + 1 < n_qb:
                    # V chains for the next block jump the queue: they gate
                    # the next block's first AV chain.
                    for t in reversed(range(4 * (qb + 1), 4 * (qb + 1) + 4)):
                        workq.appendleft(gen_v_chain(t))
                n_iters = max(4 * qb + 4, 4 * nxt[0] + 4 if nxt else 0)
                per_kt = min(6, max(2, -(-len(workq) * 9 // (n_iters * 8))))
                ao_by_qb[qb].append(
                    emit_pair((qb, hp), nxt, pT_cur, pT_next, per_kt))
            workq.append(gen_oproj(n_qb - 1, ao_by_qb.pop(n_qb - 1)))
            while workq:
                try:
                    next(workq[0])
                except StopIteration:
                    workq.popleft()

    nc.compile()
    return nc


_NC_CACHE = {}


def _get_nc(s=S):
    if s not in _NC_CACHE:
        _NC_CACHE[s] = build_nc(s)
    return _NC_CACHE[s]


def make_in_maps(x, w_q, w_k, w_v, w_o, s=S):
    """Host-side sharding: returns the 8 per-core input maps."""
    x = np.ascontiguousarray(np.asarray(x, dtype=np.float32))
    w_q = np.asarray(w_q, dtype=np.float32)
    w_k = np.asarray(w_k, dtype=np.float32)
    w_v = np.asarray(w_v, dtype=np.float32)
    w_o = np.asarray(w_o, dtype=np.float32)

    xTs = [np.ascontiguousarray(x[b].T.astype(np.float16)) for b in range(B)]
    wqTs = [np.ascontiguousarray(w_q[hg * HD:(hg + 1) * HD, :].T.astype(np.float16)) for hg in range(2)]
    wkTs = [np.ascontiguousarray(w_k[hg * HD:(hg + 1) * HD, :].T.astype(np.float16)) for hg in range(2)]
    wvTs = [np.ascontiguousarray(w_v[hg * HD:(hg + 1) * HD, :].T.astype(np.float16)) for hg in range(2)]
    woTs = [np.ascontiguousarray(w_o[:, hg * HD:(hg + 1) * HD].T.astype(np.float16)) for hg in range(2)]

    in_maps = []
    for c in range(N_CORES):
        b, hg = c // 2, c % 2
        in_maps.append({
            "xT": xTs[b], "wqT": wqTs[hg], "wkT": wkTs[hg],
            "wvT": wvTs[hg], "woT": woTs[hg],
        })
    return in_maps


def kernel(x, w_q, w_k, w_v, w_o, b_o):
    nc = _get_nc(S)
    in_maps = make_in_maps(x, w_q, w_k, w_v, w_o, s=S)
    res = run_bass_kernel_spmd(nc, in_maps, core_ids=list(range(N_CORES)))
    b_o = np.asarray(b_o, dtype=np.float32)
    outp = np.empty((B, S, D), dtype=np.float32)
    for b in range(B):
        outp[b] = res.results[2 * b]["out"] + res.results[2 * b + 1]["out"] + b_o
    return outp


# revision 22
# speedup vs baseline: 1.2032x; 1.2032x over previous
"""Multi-head causal attention (B=4, S=2048, D=1024, H=16) on 8 TRN2 NeuronCores.

Sharding: core c handles batch b = c//2 and head-group hg = c%2 (8 heads each).
Each core computes Q/K/V projections for its (batch, head-group), causal
attention, and a partial output projection over its 512 head-dims.  The host
sums the two partials per batch and adds b_o.  No collectives.

Device-side layout choices:
  - x is passed transposed (xT [D, S]) so projection matmuls contract over
    partitions directly.  x and all weights are DMA'd ONCE into SBUF with
    consolidated 3D-AP transfers and cached for the whole kernel.
  - Q and K are produced transposed (QT/KT [dq, S]); scores are computed
    transposed (S^T [kpos, q]) which makes the softmax denominator a matmul
    with a ones-column (no partition reductions anywhere).
  - V tiles are [128 tok, 9, 64]: blocks 0-7 hold the 8 heads' V, block 8 is
    a shared ones-block.  The AV stationary for head h is the strided slice
    [:, h:9:(8-h), :] = [V_h | ones], so psum rows 0-63 get U^T and rows
    64-127 the softmax denominator, at no extra PE column cost.
  - AV matmuls are causally trimmed: P^T columns q < kt*128 are identically
    zero and are never computed, exp'd, or streamed.
  - No max-subtraction in softmax: scaled scores are ~N(0,1), exp is safe.
  - Scheduling: a work queue of single-matmul "filler" steps (projection
    chains for future blocks, V chains, output projections) is drained inside
    the attention kt-loop, so the PE always has ACT-independent work while
    the scalar engine chews through the exps.
"""

import sys
import os

sys.path.insert(0, "/opt/trn_rl_repo")

from collections import deque

import numpy as np

import concourse.bacc as bacc
import concourse.mybir as mybir
import concourse.tile as tile
from concourse.bass_utils import run_bass_kernel_spmd

# The ACT table-load pass resolves each activation to the first table set
# containing it, which puts Exp (exp_and_others) and Ln
# (natural_log_exp_and_others) in different sets and reloads tables at every
# softmax normalization.  Restrict Exp/Ln to the one set that holds both so
# the whole kernel runs off a single table load.
_orig_get_tables = bacc.get_activation_tables


def _patched_tables(arch):
    t = _orig_get_tables(arch)
    for name, fns in t.items():
        if name != "natural_log_exp_and_others":
            fns.discard(mybir.ActivationFunctionType.Exp)
            fns.discard(mybir.ActivationFunctionType.Ln)
    return t


bacc.get_activation_tables = _patched_tables

B, S, D, H = 4, 2048, 1024, 16
DK = D // H          # 64
HH = H // 2          # 8 heads per core
HD = HH * DK         # 512 head-dims per core
N_CORES = 8

F32 = mybir.dt.float32
F16 = mybir.dt.float16

SCALE = 1.0 / np.sqrt(DK)


def build_nc(s=S):
    """Build the per-core SPMD program.  `s` is the sequence length (tunable
    for small-scale simulation; must be a multiple of 512)."""
    assert s % 512 == 0
    n_qb = s // 512          # 512-wide q blocks
    n_t128 = s // 128        # 128-wide token tiles
    n_dt = D // 128          # din tiles (8)
    n_dq = HD // 128         # dq tiles (4)

    nc = bacc.Bacc("TRN2", target_bir_lowering=False, debug=False,
                   num_devices=N_CORES)

    xT = nc.dram_tensor("xT", [D, s], F16, kind="ExternalInput")
    wqT = nc.dram_tensor("wqT", [D, HD], F16, kind="ExternalInput")
    wkT = nc.dram_tensor("wkT", [D, HD], F16, kind="ExternalInput")
    wvT = nc.dram_tensor("wvT", [D, HD], F16, kind="ExternalInput")
    woT = nc.dram_tensor("woT", [HD, D], F16, kind="ExternalInput")
    out = nc.dram_tensor("out", [s, D], F32, kind="ExternalOutput")

    with tile.TileContext(nc) as tc:
        with tc.tile_pool(name="persist", bufs=1) as persist, \
             tc.tile_pool(name="xc", bufs=3) as xc_pool, \
             tc.tile_pool(name="pT", bufs=40) as pT_pool, \
             tc.tile_pool(name="aoT", bufs=8) as aoT_pool, \
             tc.tile_pool(name="rb", bufs=3) as rb_pool, \
             tc.tile_pool(name="outsb", bufs=2) as out_pool, \
             tc.tile_pool(name="spsum", bufs=3, space="PSUM") as spsum, \
             tc.tile_pool(name="upsum", bufs=3, space="PSUM") as upsum, \
             tc.tile_pool(name="opsum", bufs=2, space="PSUM") as opsum:

            # Persistent SBUF arrays (live for the whole kernel).
            qt_sb = [persist.tile([128, s], F16, tag=f"qt{d}", name=f"qt{d}")
                     for d in range(n_dq)]
            # Per-head K^T tiles, zero-padded to 128 contraction rows: head h
            # occupies rows (h%2)*64..(h%2)*64+63, the other 64 rows are zero.
            # Scores matmuls can then use full 128x128 PE mode (the zero rows
            # multiply the paired head's Q rows harmlessly) -- avoiding tiling
            # mode switches, which drain the PE between instructions.
            kt_sb = [persist.tile([128, s], F16, tag=f"kt{h}", name=f"kt{h}")
                     for h in range(HH)]
            for h in range(HH):
                z = (1 - h % 2) * 64
                nc.vector.memset(kt_sb[h][z:z + 64, :], 0.0)
            # V tiles hold [t, head, 2*dk]: cols 0-63 are V, cols 64-127 are
            # 1.0.  As the AV stationary this makes the matmul emit U^T on
            # psum rows 0-63 and the softmax denominator on rows 64-127.
            v_sb = [persist.tile([128, HH, 2 * DK], F16, tag=f"v{t}",
                                 name=f"v{t}") for t in range(n_t128)]
            for t in range(n_t128):
                nc.vector.memset(v_sb[t][:, :, DK:2 * DK], 1.0)
            wq_sb = persist.tile([128, n_dt, HD], F16, tag="wq", name="wq")
            wk_sb = persist.tile([128, n_dt, HD], F16, tag="wk", name="wk")
            wv_sb = persist.tile([128, n_dt, HD], F16, tag="wv", name="wv")
            wo_sb = persist.tile([128, n_dq, D], F16, tag="wo", name="wo")

            xr = xT[:].rearrange("(i p) t -> p i t", p=128)
            wqr = wqT[:].rearrange("(i p) f -> p i f", p=128)
            wkr = wkT[:].rearrange("(i p) f -> p i f", p=128)
            wvr = wvT[:].rearrange("(i p) f -> p i f", p=128)
            wor = woT[:].rearrange("(i p) f -> p i f", p=128)

            # x block cache: 3 rotating slots (2 blocks live at a time plus
            # prefetch); tiles allocated when their DMA is issued.
            xc = {}

            def load_x_block(b):
                xc[b] = xc_pool.tile([128, n_dt, 512], F16, tag="xc",
                                     name=f"xcb{b}")
                nc.scalar.dma_start(out=xc[b][:],
                                    in_=xr[:, :, b * 512:(b + 1) * 512])

            # Startup-critical DMAs interleaved per din tile on the sync
            # queue: the first Q/K chains start as soon as their inputs land.
            xc[0] = xc_pool.tile([128, n_dt, 512], F16, tag="xc", name="xcb0")
            for i in range(n_dt):
                nc.sync.dma_start(out=xc[0][:, i, :], in_=xr[:, i, 0:512])
                nc.sync.dma_start(out=wq_sb[:, i, :], in_=wqr[:, i, :])
                nc.sync.dma_start(out=wk_sb[:, i, :], in_=wkr[:, i, :])
            # Non-critical weights + x blocks 1-2 on the ACT hwdge queue.
            nc.scalar.dma_start(out=wv_sb[:], in_=wvr)
            nc.scalar.dma_start(out=wo_sb[:], in_=wor)
            for b in (1, 2):
                if b < n_qb:
                    load_x_block(b)

            # ---- PE work generators (one matmul per yield) ----

            def gen_qk_chain(wsb, dq, tb, is_k):
                """Q^T or K^T projection for one 512-token block, one dq."""
                ps = opsum.tile([128, 512], F32, tag="op", name="pp")
                for i in range(n_dt):
                    nc.tensor.matmul(
                        ps[:],
                        lhsT=wsb[:, i, dq * 128:(dq + 1) * 128],
                        rhs=xc[tb][:, i, :],
                        start=(i == 0), stop=(i == n_dt - 1),
                    )
                    if i < n_dt - 1:
                        yield
                if is_k:
                    for e in (0, 1):
                        nc.vector.tensor_copy(
                            out=kt_sb[2 * dq + e][e * 64:(e + 1) * 64,
                                                  tb * 512:(tb + 1) * 512],
                            in_=ps[e * 64:(e + 1) * 64, :])
                else:
                    nc.vector.tensor_copy(
                        out=qt_sb[dq][:, tb * 512:(tb + 1) * 512], in_=ps[:])

            def gen_v_chain(t):
                """V projection for one 128-token tile."""
                tb, off = t // 4, (t % 4) * 128
                vp = opsum.tile([128, 512], F32, tag="op", name="vp")
                for i in range(n_dt):
                    nc.tensor.matmul(
                        vp[:],
                        lhsT=xc[tb][:, i, off:off + 128],
                        rhs=wv_sb[:, i, :],
                        start=(i == 0), stop=(i == n_dt - 1),
                    )
                    if i < n_dt - 1:
                        yield
                nc.vector.tensor_copy(
                    out=v_sb[t][:, :, 0:DK],
                    in_=vp[:].rearrange("p (h k) -> p h k", h=HH))

            def gen_oproj(qb, aos):
                """Output projection for q-block qb from its 4 ao tiles."""
                for qt_l in range(4):
                    qt = 4 * qb + qt_l
                    osb = out_pool.tile([128, D], F32, tag="osb", name="osb")
                    for half in range(2):
                        op = opsum.tile([128, 512], F32, tag="op", name="op")
                        for hp in range(n_dq):
                            nc.tensor.matmul(
                                op[:],
                                lhsT=aos[hp][:, qt_l * 128:(qt_l + 1) * 128],
                                rhs=wo_sb[:, hp, half * 512:(half + 1) * 512],
                                start=(hp == 0), stop=(hp == n_dq - 1),
                            )
                            if not (half == 1 and hp == n_dq - 1):
                                yield
                        nc.vector.tensor_copy(
                            out=osb[:, half * 512:(half + 1) * 512], in_=op[:])
                    nc.sync.dma_start(
                        out=out[qt * 128:(qt + 1) * 128, :], in_=osb[:])
                    yield

            workq = deque()

            def fill(n):
                done = 0
                while done < n and workq:
                    try:
                        next(workq[0])
                        done += 1
                    except StopIteration:
                        workq.popleft()

            def run_now(g):
                for _ in g:
                    pass

            # ---- attention ----

            def emit_score_kt(qb, hp, kt, pT):
                lo = max(kt - 4 * qb, 0) * 128
                w = 512 - lo
                for hh in (0, 1):
                    sp = spsum.tile([128, 512], F32, tag="sp", name="sp")
                    nc.tensor.matmul(
                        sp[:, lo:512],
                        lhsT=kt_sb[2 * hp + hh][:, kt * 128:(kt + 1) * 128],
                        rhs=qt_sb[hp][:, qb * 512 + lo:(qb + 1) * 512],
                        start=True, stop=True,
                    )
                    p = pT_pool.tile([128, w], F16, tag="p", name="p")
                    nc.scalar.activation(
                        out=p[:], in_=sp[:, lo:512],
                        func=mybir.ActivationFunctionType.Exp,
                        scale=float(SCALE))
                    if kt >= 4 * qb:
                        # zero strict-upper (kpos > q) region of the
                        # diagonal-crossing tile
                        nc.gpsimd.affine_select(
                            out=p[:], in_=p[:],
                            compare_op=mybir.AluOpType.is_ge,
                            fill=0.0, base=0, channel_multiplier=-1,
                            pattern=[[1, w]])
                    pT[(kt, hh)] = (p, lo)

            def emit_pair(cur, nxt, pT_cur, pT_next, per_kt):
                """Interleave next pair's scores with current pair's AV
                chains at kt granularity, plus `per_kt` filler steps: the PE
                gets dependency-free matmuls to run while the ACT engine
                works through the scores' exps."""
                nkt_cur = 4 * cur[0] + 4 if cur else 0
                nkt_nxt = 4 * nxt[0] + 4 if nxt else 0
                u = {}
                ao = None
                if cur:
                    ao = aoT_pool.tile([128, 512], F16, tag="aoT", name="aoT")
                    for hh in (0, 1):
                        u[hh] = upsum.tile([128, 512], F32, tag="u", name="u")
                for kt in range(max(nkt_cur, nkt_nxt)):
                    if kt < nkt_nxt:
                        emit_score_kt(nxt[0], nxt[1], kt, pT_next)
                    if kt == 0:
                        # fillers first: the pair's first AV matmul may wait
                        # on the u-slot WAR (previous pair's normalize chain);
                        # let the PE chew fillers during that window.
                        fill(per_kt)
                    if kt < nkt_cur:
                        for hh in (0, 1):
                            p, lo = pT_cur[(kt, hh)]
                            nc.tensor.matmul(
                                u[hh][:, lo:512],
                                lhsT=v_sb[kt][:, 2 * cur[1] + hh, :],
                                rhs=p[:],
                                start=(kt == 0), stop=(kt == nkt_cur - 1),
                            )
                    if kt > 0:
                        fill(per_kt)
                if cur:
                    # Both heads' denominators (u rows 64-127) staged into one
                    # rb tile so a single Ln+Exp pass computes 1/l for both.
                    # 1/l = exp(-ln(l)): ln and exp share one ACT table set,
                    # so no table reloads.
                    rb = rb_pool.tile([128, 512], F32, tag="rb", name="rb")
                    for hh in (0, 1):
                        nc.vector.tensor_copy(
                            out=rb[hh * 64:(hh + 1) * 64, :],
                            in_=u[hh][64:128, :])
                    nc.scalar.activation(
                        out=rb[:], in_=rb[:],
                        func=mybir.ActivationFunctionType.Ln)
                    nc.scalar.activation(
                        out=rb[:], in_=rb[:],
                        func=mybir.ActivationFunctionType.Exp, scale=-1.0)
                    for hh in (0, 1):
                        nc.vector.tensor_mul(
                            out=ao[hh * 64:(hh + 1) * 64, :],
                            in0=u[hh][0:64, :], in1=rb[hh * 64:(hh + 1) * 64, :])
                return ao

            # ---- schedule ----
            # Pre-loop: block-0 Q/K chains (dq-interleaved so pair (0,0)
            # unblocks after two chains), pair-(0,0) scores, block-0 V
            # chains, block-1 Q/K chains.  Everything later flows through
            # the filler queue.
            for dq in range(n_dq):
                run_now(gen_qk_chain(wq_sb, dq, 0, False))
                run_now(gen_qk_chain(wk_sb, dq, 0, True))
            pairs = [(qb, hp) for qb in range(n_qb) for hp in range(n_dq)]
            pT_next = {}
            emit_pair(None, pairs[0], None, pT_next, 0)
            for t in range(min(4, n_t128)):
                run_now(gen_v_chain(t))
            if n_qb > 1:
                for dq in range(n_dq):
                    run_now(gen_qk_chain(wq_sb, dq, 1, False))
                    run_now(gen_qk_chain(wk_sb, dq, 1, True))

            ao_by_qb = {qb: [] for qb in range(n_qb)}
            for i, (qb, hp) in enumerate(pairs):
                pT_cur, pT_next = pT_next, {}
                nxt = pairs[i + 1] if i + 1 < len(pairs) else None
                if hp == 0:
                    if qb + 3 < n_qb:
                        load_x_block(qb + 3)
                    if qb > 0:
                        workq.append(gen_oproj(qb - 1, ao_by_qb.pop(qb - 1)))
                    if qb + 2 < n_qb:
                        for dq in range(n_dq):
                            workq.append(gen_qk_chain(wq_sb, dq, qb + 2, False))
                            workq.append(gen_qk_chain(wk_sb, dq, qb + 2, True))
                if hp == 2 and qb + 1 < n_qb:
                    # V chains for the next block jump the queue: they gate
                    # the next block's first AV chain.
                    for t in reversed(range(4 * (qb + 1), 4 * (qb + 1) + 4)):
                        workq.appendleft(gen_v_chain(t))
                n_iters = max(4 * qb + 4, 4 * nxt[0] + 4 if nxt else 0)
                per_kt = min(6, max(2, -(-len(workq) * 9 // (n_iters * 8))))
                ao_by_qb[qb].append(
                    emit_pair((qb, hp), nxt, pT_cur, pT_next, per_kt))
            workq.append(gen_oproj(n_qb - 1, ao_by_qb.pop(n_qb - 1)))
            while workq:
                try:
                    next(workq[0])
                except StopIteration:
                    workq.popleft()

    nc.compile()
    return nc


_NC_CACHE = {}


def _get_nc(s=S):
    if s not in _NC_CACHE:
        _NC_CACHE[s] = build_nc(s)
    return _NC_CACHE[s]


def make_in_maps(x, w_q, w_k, w_v, w_o, s=S):
    """Host-side sharding: returns the 8 per-core input maps."""
    x = np.ascontiguousarray(np.asarray(x, dtype=np.float32))
    w_q = np.asarray(w_q, dtype=np.float32)
    w_k = np.asarray(w_k, dtype=np.float32)
    w_v = np.asarray(w_v, dtype=np.float32)
    w_o = np.asarray(w_o, dtype=np.float32)

    xTs = [np.ascontiguousarray(x[b].T.astype(np.float16)) for b in range(B)]
    wqTs = [np.ascontiguousarray(w_q[hg * HD:(hg + 1) * HD, :].T.astype(np.float16)) for hg in range(2)]
    wkTs = [np.ascontiguousarray(w_k[hg * HD:(hg + 1) * HD, :].T.astype(np.float16)) for hg in range(2)]
    wvTs = [np.ascontiguousarray(w_v[hg * HD:(hg + 1) * HD, :].T.astype(np.float16)) for hg in range(2)]
    woTs = [np.ascontiguousarray(w_o[:, hg * HD:(hg + 1) * HD].T.astype(np.float16)) for hg in range(2)]

    in_maps = []
    for c in range(N_CORES):
        b, hg = c // 2, c % 2
        in_maps.append({
            "xT": xTs[b], "wqT": wqTs[hg], "wkT": wkTs[hg],
            "wvT": wvTs[hg], "woT": woTs[hg],
        })
    return in_maps


def kernel(x, w_q, w_k, w_v, w_o, b_o):
    nc = _get_nc(S)
    in_maps = make_in_maps(x, w_q, w_k, w_v, w_o, s=S)
    res = run_bass_kernel_spmd(nc, in_maps, core_ids=list(range(N_CORES)))
    b_o = np.asarray(b_o, dtype=np.float32)
    outp = np.empty((B, S, D), dtype=np.float32)
    for b in range(B):
        outp[b] = res.results[2 * b]["out"] + res.results[2 * b + 1]["out"] + b_o
    return outp
